# revision 1
# baseline (speedup 1.0000x reference)
"""ARMA-style GNN message passing on 8 TRN2 NeuronCores.

Reference computation (per layer, 7 layers):
    m   = h @ W                                  [N, CH]
    agg = segment_sum(w[:,None] * m[dst], src)   [N, CH]
    h'  = relu(agg + h @ V + b)
then logits = h @ Wd + bd.

Strategy (graph/data parallel over nodes):
  - 8 cores own 1250 nodes each (padded to 1280 = 10 blocks of 128).
  - Edge (s, d) is processed by the core owning s (the aggregation target).
    Host sorts each core's edges into its src node-blocks, pads each block's
    edge list to a multiple of 128, and builds per-edge-block:
      * gather indices (padded-global row of d in the all-gathered m table)
      * a [128 edges x 128 nodes] bf16 "selection" matrix carrying the
        degree weights w_e -- segment-sum becomes sel.T @ gathered_rows on PE.
  - Per layer: each core computes m for its own nodes (PE), AllGathers m
    (bf16) so every core has the full table in DRAM, then per edge block
    one indirect DMA gathers 128 rows (1 KB each) and one matmul
    scatter-adds them into the PSUM accumulator of the owning node block.
    h@V accumulates into the same PSUM bank; bias + relu + PE-transpose
    produce the next layer's stationary operand hT.
  - Final dense layer and output assembly per core; host concatenates.

All matmuls run in bf16 with fp32 PSUM accumulation.
"""
import numpy as np
import ml_dtypes

import concourse.bass as bass
import concourse.tile as tile
import concourse.mybir as mybir
from concourse.vector_clock import ScopedClock
from concourse.bass_utils import run_bass_kernel_spmd
from concourse.masks import make_identity

# ---------------------------------------------------------------- constants
N_NODES = 10000
N_EDGES = 160000
IN_F = 256
CH = 512
N_LABELS = 1440
NCORES = 8
NPC = N_NODES // NCORES      # 1250 nodes per core
P = 128
NBL = 10                     # node blocks per core (10*128 = 1280)
NPAD = NBL * P               # padded nodes per core
NLAYERS = 7
KG1 = IN_F // P              # 2 contraction blocks in layer 1
KGC = CH // P                # 4 contraction blocks in layers 2..7
FIN_CHUNK = 480              # 1440 = 3 * 480, fits one PSUM bank in f32

BF = mybir.dt.bfloat16
F32 = mybir.dt.float32
BFNP = ml_dtypes.bfloat16


# ------------------------------------------------------- walrus workarounds
def _patched_drain_and_barrier(self, tick_clock, wait_clock):
    # This walrus build rejects >1-2 sync waits on one TPB_CTRL; put the
    # kernel-tail drain's waits on separate preceding SP nops instead.
    nc = self.nc
    probe = nc.sync.nop(nofuse=True, hint="drain_waits")
    wait_clock.add_sem_waits(probe.ins, ScopedClock({None: tick_clock.global_clock}))
    si = probe.ins.sync_info
    waits = list(si.on_wait) if si is not None else []
    if len(waits) > 1:
        si.on_wait = waits[:1]
        for i in range(1, len(waits)):
            n2 = nc.sync.nop(nofuse=True, hint=f"drain_waits_{i}")
            n2.ins.sync_info = mybir.SyncInfo(on_wait=[waits[i]], on_update=[])
    nc.sync.drain()
    nc.all_engine_barrier()
    assert self.sems is not None
    popped = nc._tile_sem_poison_stack.pop()
    assert popped is self._sem_poison
    nc.clear_and_free_semaphores(list(self.sems.allocated().values()))
    nc.all_engine_barrier()


tile.TileContext._drain_and_barrier = _patched_drain_and_barrier


def _split_excess_waits(nc, limit=1):
    # Same ISA restriction for ordinary instructions: hoist excess sync
    # waits onto injected same-engine nops placed just before.
    for func in nc.m.functions:
        for bb in func.blocks:
            out = []
            for ins in bb.instructions:
                si = ins.sync_info
                if si is not None and si.on_wait and len(si.on_wait) > limit:
                    waits = list(si.on_wait)
                    excess, keep = waits[:-limit], waits[-limit:]
                    for i in range(0, len(excess), limit):
                        out.append(mybir.InstNoOp(
                            name=f"{ins.name}_xw{i}",
                            engine=ins.engine,
                            ins=[], outs=[],
                            sync_info=mybir.SyncInfo(
                                on_wait=excess[i:i + limit], on_update=[]),
                        ))
                    si.on_wait = keep
                out.append(ins)
            bb.instructions[:] = out


# ------------------------------------------------------------- host prep
def _prep_edges(src, dst):
    """Partition/sort edges by owning core of src; build per-core gather
    index tables and selection matrices. Returns (knb, idx_tabs, sel_tabs)
    where knb[nb] = edge-block count of node block nb (same on all cores)."""
    src = np.asarray(src).astype(np.int64)
    dst = np.asarray(dst).astype(np.int64)
    deg_out = np.maximum(np.bincount(src, minlength=N_NODES), 1.0).astype(np.float32)
    deg_in = np.maximum(np.bincount(dst, minlength=N_NODES), 1.0).astype(np.float32)
    w = 1.0 / np.sqrt(deg_out[src] * deg_in[dst])

    core = src // NPC
    loc = src - core * NPC
    nb = loc // P
    ncol = loc - nb * P          # column within the node block's sel matrix
    grow = (dst // NPC) * NPAD + (dst % NPC)   # padded-global gather row

    # bucket edges by (core, nb)
    order = np.lexsort((nb, core))
    core_s, nb_s = core[order], nb[order]
    ncol_s, grow_s, w_s = ncol[order], grow[order], w[order]
    counts = np.zeros((NCORES, NBL), np.int64)
    np.add.at(counts, (core_s, nb_s), 1)
    knb = [max(1, int(-(-counts[:, b].max() // P))) for b in range(NBL)]
    neb = sum(knb)

    idx_tabs, sel_tabs = [], []
    starts = np.zeros((NCORES, NBL), np.int64)
    flat = counts.ravel().cumsum()
    starts.ravel()[1:] = flat[:-1]
    for p in range(NCORES):
        idx_t = np.zeros((P, neb), np.int32)
        sel_t = np.zeros((P, neb * P), np.float32)
        col = 0
        for b in range(NBL):
            s0, cnt = starts[p, b], counts[p, b]
            g = grow_s[s0:s0 + cnt]
            c = ncol_s[s0:s0 + cnt]
            ww = w_s[s0:s0 + cnt]
            for k in range(knb[b]):
                lo, hi = k * P, min((k + 1) * P, cnt)
                if hi > lo:
                    lanes = np.arange(hi - lo)
                    idx_t[lanes, col] = g[lo:hi]
                    sel_t[lanes, col * P + c[lo:hi]] = ww[lo:hi]
                col += 1
        idx_tabs.append(idx_t)
        sel_tabs.append(sel_t.astype(BFNP))
    return knb, idx_tabs, sel_tabs


def _pack_lhsT(xT, kg):
    """[kg*128, NPAD] -> [128, kg*NPAD] (partition-major kg blocks)."""
    return np.ascontiguousarray(
        xT.reshape(kg, P, NPAD).transpose(1, 0, 2).reshape(P, kg * NPAD))


def _pack_rhs(Wm, kg, n):
    """[kg*128, n] -> [128, kg*n]."""
    return np.ascontiguousarray(
        Wm.reshape(kg, P, n).transpose(1, 0, 2).reshape(P, kg * n))


# ------------------------------------------------------------- device build
def _build(knb, repeat=1):
    neb = sum(knb)
    nc = bass.Bass("TRN2", target_bir_lowering=False, debug=False,
                   num_devices=NCORES)

    def din(name, shape, dt):
        return nc.dram_tensor(name, shape, dt, kind="ExternalInput").ap()

    xT = din("xT", [P, KG1 * NPAD], BF)
    idx = din("idx", [P, neb], mybir.dt.int32)
    sel = din("sel", [P, neb * P], BF)
    w1 = din("w1", [P, KG1 * CH], BF)
    v1 = din("v1", [P, KG1 * CH], BF)
    wk = din("wk", [P, 6 * KGC * CH], BF)
    vk = din("vk", [P, 6 * KGC * CH], BF)
    wd = din("wd", [P, KGC * N_LABELS], BF)
    ball = din("ball", [P, NLAYERS * CH], F32)
    bdr = din("bdr", [P, N_LABELS], F32)
    out = nc.dram_tensor("out", [NPAD, N_LABELS], F32, kind="ExternalOutput").ap()

    with tile.TileContext(nc) as tc:
        with (
            tc.tile_pool(name="const", bufs=1) as cp,
            tc.tile_pool(name="ht", bufs=2) as htp,
            tc.tile_pool(name="mout", bufs=3) as mp,
            tc.tile_pool(name="msg", bufs=16) as msgp,
            tc.tile_pool(name="hact", bufs=2) as hp,
            tc.tile_pool(name="outs", bufs=2) as op,
            tc.tile_pool(name="psm", bufs=2, space="PSUM") as psm,
            tc.tile_pool(name="psagg", bufs=4, space="PSUM") as psagg,
            tc.tile_pool(name="pstr", bufs=2, space="PSUM") as pstr,
            tc.tile_pool(name="dram", bufs=1, space="DRAM") as dram,
        ):
            # ---- constants to SBUF
            xT_t = cp.tile([P, KG1 * NPAD], BF)
            nc.sync.dma_start(xT_t[:], xT[:])
            idx_t = cp.tile([P, neb], mybir.dt.int32)
            nc.sync.dma_start(idx_t[:], idx[:])
            sel_t = cp.tile([P, neb * P], BF)
            nc.sync.dma_start(sel_t[:], sel[:])
            w1_t = cp.tile([P, KG1 * CH], BF)
            nc.sync.dma_start(w1_t[:], w1[:])
            v1_t = cp.tile([P, KG1 * CH], BF)
            nc.sync.dma_start(v1_t[:], v1[:])
            wk_t = cp.tile([P, 6 * KGC * CH], BF)
            nc.sync.dma_start(wk_t[:], wk[:])
            vk_t = cp.tile([P, 6 * KGC * CH], BF)
            nc.sync.dma_start(vk_t[:], vk[:])
            wd_t = cp.tile([P, KGC * N_LABELS], BF)
            nc.sync.dma_start(wd_t[:], wd[:])
            ball_t = cp.tile([P, NLAYERS * CH], F32)
            nc.sync.dma_start(ball_t[:], ball[:])
            bdr_t = cp.tile([P, N_LABELS], F32)
            nc.sync.dma_start(bdr_t[:], bdr[:])
            ident = cp.tile([P, P], BF)
            make_identity(nc, ident[:])

            for rep in range(repeat):
                hT_cur = None
                for l in range(NLAYERS):
                    kg = KG1 if l == 0 else KGC
                    if l == 0:
                        lhsT_t, lw = xT_t, NPAD * KG1
                        wt = w1_t[:, :]
                        vt = v1_t[:, :]
                    else:
                        lhsT_t, lw = hT_cur, NPAD * KGC
                        wt = wk_t[:, (l - 1) * KGC * CH:l * KGC * CH]
                        vt = vk_t[:, (l - 1) * KGC * CH:l * KGC * CH]

                    # --- m = h @ W for own nodes; stage to DRAM for AllGather
                    ag_in = dram.tile([NPAD, CH], BF, tag="ag_in")
                    for b in range(NBL):
                        m_ps = psm.tile([P, CH], F32, tag="m")
                        for g in range(kg):
                            nc.tensor.matmul(
                                m_ps[:],
                                lhsT_t[:, g * NPAD + b * P:g * NPAD + (b + 1) * P],
                                wt[:, g * CH:(g + 1) * CH],
                                start=(g == 0), stop=(g == kg - 1))
                        m_bf = mp.tile([P, CH], BF, tag="mbf")
                        nc.vector.tensor_copy(m_bf[:], m_ps[:])
                        nc.sync.dma_start(ag_in[b * P:(b + 1) * P, :], m_bf[:])

                    ag_out = dram.tile([NCORES * NPAD, CH], BF,
                                       tag=f"ag_out{l}", addr_space="Shared")
                    nc.gpsimd.collective_compute(
                        "AllGather", mybir.AluOpType.bypass,
                        replica_groups=[list(range(NCORES))],
                        ins=[ag_in[:].opt()], outs=[ag_out[:].opt()])

                    # --- per node block: hV + scattered messages -> h'
                    hT_next = htp.tile([P, KGC * NPAD], BF, tag="hT")
                    col = 0
                    for b in range(NBL):
                        h_ps = psagg.tile([P, CH], F32, tag="agg")
                        for g in range(kg):
                            nc.tensor.matmul(
                                h_ps[:],
                                lhsT_t[:, g * NPAD + b * P:g * NPAD + (b + 1) * P],
                                vt[:, g * CH:(g + 1) * CH],
                                start=(g == 0), stop=False)
                        for k in range(knb[b]):
                            msg = msgp.tile([P, CH], BF, tag="msg")
                            nc.gpsimd.indirect_dma_start(
                                out=msg[:], out_offset=None,
                                in_=ag_out[:],
                                in_offset=bass.IndirectOffsetOnAxis(
                                    ap=idx_t[:, col:col + 1], axis=0))
                            nc.tensor.matmul(
                                h_ps[:],
                                sel_t[:, col * P:(col + 1) * P],
                                msg[:],
                                start=False, stop=(k == knb[b] - 1))
                            col += 1
                        nc.vector.tensor_add(
                            h_ps[:], h_ps[:], ball_t[:, l * CH:(l + 1) * CH])
                        h_bf = hp.tile([P, CH], BF, tag="h")
                        nc.scalar.activation(
                            h_bf[:], h_ps[:], mybir.ActivationFunctionType.Relu)
                        for cg in range(KGC):
                            tr_ps = pstr.tile([P, P], BF, tag="tr")
                            nc.tensor.transpose(
                                tr_ps[:], h_bf[:, cg * P:(cg + 1) * P], ident[:])
                            nc.vector.tensor_copy(
                                hT_next[:, cg * NPAD + b * P:cg * NPAD + (b + 1) * P],
                                tr_ps[:])
                    hT_cur = hT_next

                # ---- final dense: logits = h7 @ Wd + bd
                for b in range(NBL):
                    o_sb = op.tile([P, N_LABELS], F32, tag="o")
                    fps = []
                    for c in range(3):
                        fin_ps = psagg.tile([P, FIN_CHUNK], F32, tag="agg")
                        fps.append(fin_ps)
                    for g in range(KGC):
                        for c in range(3):
                            nc.tensor.matmul(
                                fps[c][:],
                                hT_cur[:, g * NPAD + b * P:g * NPAD + (b + 1) * P],
                                wd_t[:, g * N_LABELS + c * FIN_CHUNK:
                                     g * N_LABELS + (c + 1) * FIN_CHUNK],
                                start=(g == 0), stop=(g == KGC - 1))
                    for c in range(3):
                        sl = slice(c * FIN_CHUNK, (c + 1) * FIN_CHUNK)
                        nc.vector.tensor_add(fps[c][:], fps[c][:], bdr_t[:, sl])
                        nc.scalar.activation(
                            o_sb[:, sl], fps[c][:],
                            mybir.ActivationFunctionType.Copy)
                    if rep == repeat - 1:
                        nc.sync.dma_start(out[b * P:(b + 1) * P, :], o_sb[:])

    _split_excess_waits(nc)
    return nc


# ------------------------------------------------------------- entry point
def kernel(x, src, dst, W1, V1, b1, Wk, Vk, bk, Wd, bd, _repeat=1, _nc_cache={}):
    x = np.asarray(x, np.float32)
    knb, idx_tabs, sel_tabs = _prep_edges(src, dst)

    key = (tuple(knb), _repeat)
    if key not in _nc_cache:
        _nc_cache[key] = _build(knb, repeat=_repeat)
    nc = _nc_cache[key]

    # weights (replicated, host-packed)
    w1p = _pack_rhs(np.asarray(W1, np.float32), KG1, CH).astype(BFNP)
    v1p = _pack_rhs(np.asarray(V1, np.float32), KG1, CH).astype(BFNP)
    wkp = np.concatenate(
        [_pack_rhs(np.asarray(Wk[i], np.float32), KGC, CH) for i in range(6)],
        axis=1).astype(BFNP)
    vkp = np.concatenate(
        [_pack_rhs(np.asarray(Vk[i], np.float32), KGC, CH) for i in range(6)],
        axis=1).astype(BFNP)
    wdp = _pack_rhs(np.asarray(Wd, np.float32), KGC, N_LABELS).astype(BFNP)
    ballv = np.concatenate(
        [np.asarray(b1, np.float32)] + [np.asarray(bk[i], np.float32)
                                        for i in range(6)])
    ballp = np.broadcast_to(ballv, (P, NLAYERS * CH)).copy()
    bdp = np.broadcast_to(np.asarray(bd, np.float32), (P, N_LABELS)).copy()

    in_maps = []
    for p in range(NCORES):
        xp = np.zeros((NPAD, IN_F), np.float32)
        xp[:NPC] = x[p * NPC:(p + 1) * NPC]
        xTp = _pack_lhsT(np.ascontiguousarray(xp.T), KG1).astype(BFNP)
        in_maps.append({
            "xT": xTp, "idx": idx_tabs[p], "sel": sel_tabs[p],
            "w1": w1p, "v1": v1p, "wk": wkp, "vk": vkp, "wd": wdp,
            "ball": ballp, "bdr": bdp,
        })

    res = run_bass_kernel_spmd(nc, in_maps, core_ids=list(range(NCORES)))
    outp = np.empty((N_NODES, N_LABELS), np.float32)
    for p in range(NCORES):
        outp[p * NPC:(p + 1) * NPC] = res.results[p]["out"][:NPC]
    return outp



# revision 3
# speedup vs baseline: 1.1220x; 1.1220x over previous
"""ARMA-style GNN message passing on 8 TRN2 NeuronCores.

Reference computation (per layer, 7 layers):
    m   = h @ W                                  [N, CH]
    agg = segment_sum(w[:,None] * m[dst], src)   [N, CH]
    h'  = relu(agg + h @ V + b)
then logits = h @ Wd + bd.

Strategy (graph/data parallel over nodes, ReduceScatter aggregation):
  - 8 cores own 1250 nodes each (padded to 1280 = 10 blocks of 128).
  - Edge (s, d) is processed by the core owning d (the message SOURCE row):
    core q computes m for its own nodes only, keeps it local, and produces
    PARTIAL aggregates for ALL 80 global src node-blocks from its own edges.
    A ReduceScatter(add) then sums the 8 partials and hands each core the
    aggregate rows for its own nodes.  Output of the collective is only
    [1280, 512] bf16 per core (vs a 10.5 MB AllGather), and all message
    gathers are core-local.
  - Per global src block b, the host dedups the edges' dst rows (unique
    rows only), builds gather index chunks of 128 rows and [128 x 128]
    bf16 "C" matrices with the summed degree weights; segment-sum becomes
    C.T @ gathered_rows on PE, accumulated in PSUM over the block's chunks.
  - Gathers are batched (8 chunks per indirect DMA) to amortize the SWDGE
    fixed overhead.
  - h@V accumulates into the same PSUM bank as the scattered aggregate
    (identity-matmul injects the ReduceScatter result); bias + relu +
    PE-transpose produce the next layer's stationary operand hT.
  - Final dense layer and output assembly per core; host concatenates.

All matmuls run in bf16 with fp32 PSUM accumulation.
"""
import numpy as np
import ml_dtypes

import concourse.bass as bass
import concourse.tile as tile
import concourse.mybir as mybir
from concourse.vector_clock import ScopedClock
from concourse.bass_utils import run_bass_kernel_spmd
from concourse.masks import make_identity

# ---------------------------------------------------------------- constants
N_NODES = 10000
N_EDGES = 160000
IN_F = 256
CH = 512
N_LABELS = 1440
NCORES = 8
NPC = N_NODES // NCORES      # 1250 nodes per core
P = 128
NBL = 10                     # node blocks per core (10*128 = 1280)
NPAD = NBL * P               # padded nodes per core
GBL = NCORES * NBL           # 80 global src node blocks
NLAYERS = 7
KG1 = IN_F // P              # 2 contraction blocks in layer 1
KGC = CH // P                # 4 contraction blocks in layers 2..7
FIN_CHUNK = 480              # 1440 = 3 * 480, fits one PSUM bank in f32
GBATCH = 1                   # gather chunks batched per indirect DMA

BF = mybir.dt.bfloat16
F32 = mybir.dt.float32
BFNP = ml_dtypes.bfloat16

# processing order of global src blocks: j-major so that each core's
# low-j blocks complete first (enables split collectives)
GB_ORDER = [p * NBL + j for j in range(NBL) for p in range(NCORES)]


# ------------------------------------------------------- walrus workarounds
def _patched_drain_and_barrier(self, tick_clock, wait_clock):
    # This walrus build rejects >1-2 sync waits on one TPB_CTRL; put the
    # kernel-tail drain's waits on separate preceding SP nops instead.
    nc = self.nc
    probe = nc.sync.nop(nofuse=True, hint="drain_waits")
    wait_clock.add_sem_waits(probe.ins, ScopedClock({None: tick_clock.global_clock}))
    si = probe.ins.sync_info
    waits = list(si.on_wait) if si is not None else []
    if len(waits) > 1:
        si.on_wait = waits[:1]
        for i in range(1, len(waits)):
            n2 = nc.sync.nop(nofuse=True, hint=f"drain_waits_{i}")
            n2.ins.sync_info = mybir.SyncInfo(on_wait=[waits[i]], on_update=[])
    nc.sync.drain()
    nc.all_engine_barrier()
    assert self.sems is not None
    popped = nc._tile_sem_poison_stack.pop()
    assert popped is self._sem_poison
    nc.clear_and_free_semaphores(list(self.sems.allocated().values()))
    nc.all_engine_barrier()


tile.TileContext._drain_and_barrier = _patched_drain_and_barrier


def _split_excess_waits(nc, limit=1):
    # Same ISA restriction for ordinary instructions: hoist excess sync
    # waits onto injected same-engine nops placed just before.
    for func in nc.m.functions:
        for bb in func.blocks:
            out = []
            for ins in bb.instructions:
                si = ins.sync_info
                if si is not None and si.on_wait and len(si.on_wait) > limit:
                    waits = list(si.on_wait)
                    excess, keep = waits[:-limit], waits[-limit:]
                    for i in range(0, len(excess), limit):
                        out.append(mybir.InstNoOp(
                            name=f"{ins.name}_xw{i}",
                            engine=ins.engine,
                            ins=[], outs=[],
                            sync_info=mybir.SyncInfo(
                                on_wait=excess[i:i + limit], on_update=[]),
                        ))
                    si.on_wait = keep
                out.append(ins)
            bb.instructions[:] = out


# ------------------------------------------------------------- host prep
def _prep_edges(src, dst):
    """Partition edges by owning core of dst; per (core, global src block)
    dedup dst rows and build gather-index chunks + C matrices (summed
    degree weights).  Returns (kcb, idx_tabs, c_tabs): kcb[i] = chunk count
    of the i-th block in GB_ORDER (same on all cores)."""
    src = np.asarray(src).astype(np.int64)
    dst = np.asarray(dst).astype(np.int64)
    deg_out = np.maximum(np.bincount(src, minlength=N_NODES), 1.0).astype(np.float64)
    deg_in = np.maximum(np.bincount(dst, minlength=N_NODES), 1.0).astype(np.float64)
    w = (1.0 / np.sqrt(deg_out[src] * deg_in[dst])).astype(np.float32)

    core = dst // NPC                       # owner of the message source row
    ldst = dst - core * NPC                 # local gather row
    sloc = src % NPC
    gb = (src // NPC) * NBL + sloc // P     # global src block
    slot = sloc % P                         # column within the block

    # bucket edges by (core, gb)
    order = np.lexsort((gb, core))
    core_s, gb_s = core[order], gb[order]
    ldst_s, slot_s, w_s = ldst[order], slot[order], w[order]
    counts = np.zeros((NCORES, GBL), np.int64)
    np.add.at(counts, (core_s, gb_s), 1)
    starts = np.zeros((NCORES, GBL), np.int64)
    flat = counts.ravel().cumsum()
    starts.ravel()[1:] = flat[:-1]

    # unique-dst rows per (core, gb)
    uniq = [[None] * GBL for _ in range(NCORES)]
    nuniq = np.zeros((NCORES, GBL), np.int64)
    inv_all = [[None] * GBL for _ in range(NCORES)]
    for q in range(NCORES):
        for b in range(GBL):
            s0, cnt = starts[q, b], counts[q, b]
            u, inv = np.unique(ldst_s[s0:s0 + cnt], return_inverse=True)
            uniq[q][b] = u
            inv_all[q][b] = inv
            nuniq[q, b] = len(u)

    kcb = [max(1, int(-(-nuniq[:, b].max() // P))) for b in GB_ORDER]
    neb = sum(kcb)

    idx_tabs, c_tabs = [], []
    for q in range(NCORES):
        idx_t = np.zeros((P, neb), np.int32)
        c_t = np.zeros((P, neb * P), np.float32)
        col = 0
        for oi, b in enumerate(GB_ORDER):
            s0, cnt = starts[q, b], counts[q, b]
            u = uniq[q][b]
            inv = inv_all[q][b]          # edge -> position in u
            sl = slot_s[s0:s0 + cnt]
            ww = w_s[s0:s0 + cnt]
            for k in range(kcb[oi]):
                lo, hi = k * P, min((k + 1) * P, len(u))
                if hi > lo:
                    idx_t[:hi - lo, col] = u[lo:hi]
                    sel = (inv >= lo) & (inv < hi)
                    np.add.at(c_t, (inv[sel] - lo, col * P + sl[sel]), ww[sel])
                col += 1
        idx_tabs.append(idx_t)
        c_tabs.append(c_t.astype(BFNP))
    return kcb, idx_tabs, c_tabs


def _pack_lhsT(xT, kg):
    """[kg*128, NPAD] -> [128, kg*NPAD] (partition-major kg blocks)."""
    return np.ascontiguousarray(
        xT.reshape(kg, P, NPAD).transpose(1, 0, 2).reshape(P, kg * NPAD))


def _pack_rhs(Wm, kg, n):
    """[kg*128, n] -> [128, kg*n]."""
    return np.ascontiguousarray(
        Wm.reshape(kg, P, n).transpose(1, 0, 2).reshape(P, kg * n))


# ------------------------------------------------------------- device build
def _build(kcb, repeat=1):
    neb = sum(kcb)
    nc = bass.Bass("TRN2", target_bir_lowering=False, debug=False,
                   num_devices=NCORES)

    def din(name, shape, dt):
        return nc.dram_tensor(name, shape, dt, kind="ExternalInput").ap()

    xT = din("xT", [P, KG1 * NPAD], BF)
    idx = din("idx", [P, neb], mybir.dt.int32)
    ctab = din("ctab", [P, neb * P], BF)
    w1 = din("w1", [P, KG1 * CH], BF)
    v1 = din("v1", [P, KG1 * CH], BF)
    wk = din("wk", [P, 6 * KGC * CH], BF)
    vk = din("vk", [P, 6 * KGC * CH], BF)
    wd = din("wd", [P, KGC * N_LABELS], BF)
    ball = din("ball", [P, NLAYERS * CH], F32)
    bdr = din("bdr", [P, N_LABELS], F32)
    out = nc.dram_tensor("out", [NPAD, N_LABELS], F32, kind="ExternalOutput").ap()

    # chunk -> ordered-block mapping
    chunk_block = []          # index into GB_ORDER position for each chunk col
    for oi in range(GBL):
        chunk_block.extend([oi] * kcb[oi])

    with tile.TileContext(nc) as tc:
        with (
            tc.tile_pool(name="const", bufs=1) as cp,
            tc.tile_pool(name="ht", bufs=2) as htp,
            tc.tile_pool(name="mout", bufs=3) as mp,
            tc.tile_pool(name="msg", bufs=2) as msgp,
            tc.tile_pool(name="aggf", bufs=4) as aggfp,
            tc.tile_pool(name="hact", bufs=2) as hp,
            tc.tile_pool(name="outs", bufs=3) as op,
            tc.tile_pool(name="psm", bufs=2, space="PSUM") as psm,
            tc.tile_pool(name="psagg", bufs=4, space="PSUM") as psagg,
            tc.tile_pool(name="pstr", bufs=2, space="PSUM") as pstr,
            tc.tile_pool(name="dram", bufs=1, space="DRAM") as dram,
        ):
            # ---- constants needed for layer 0 first
            xT_t = cp.tile([P, KG1 * NPAD], BF)
            nc.sync.dma_start(xT_t[:], xT[:])
            w1_t = cp.tile([P, KG1 * CH], BF)
            nc.sync.dma_start(w1_t[:], w1[:])
            v1_t = cp.tile([P, KG1 * CH], BF)
            nc.sync.dma_start(v1_t[:], v1[:])
            idx_t = cp.tile([P, neb], mybir.dt.int32)
            nc.sync.dma_start(idx_t[:], idx[:])
            ctab_t = cp.tile([P, neb * P], BF)
            nc.sync.dma_start(ctab_t[:], ctab[:])
            ball_t = cp.tile([P, NLAYERS * CH], F32)
            nc.sync.dma_start(ball_t[:], ball[:])
            ident = cp.tile([P, P], BF)
            make_identity(nc, ident[:])
            # late constants (first needed at layer 1 / final layer)
            wk_t = cp.tile([P, 6 * KGC * CH], BF)
            vk_t = cp.tile([P, 6 * KGC * CH], BF)
            wd_t = cp.tile([P, KGC * N_LABELS], BF)
            bdr_t = cp.tile([P, N_LABELS], F32)

            for rep in range(repeat):
                hT_cur = None
                for l in range(NLAYERS):
                    kg = KG1 if l == 0 else KGC
                    if l == 0:
                        lhsT_t = xT_t
                        wt = w1_t[:, :]
                        vt = v1_t[:, :]
                    else:
                        lhsT_t = hT_cur
                        wt = wk_t[:, (l - 1) * KGC * CH:l * KGC * CH]
                        vt = vk_t[:, (l - 1) * KGC * CH:l * KGC * CH]

                    # --- m = h @ W for own nodes; stage to local DRAM
                    m_dram = dram.tile([NPAD, CH], BF, tag="m_dram")
                    for b in range(NBL):
                        m_ps = psm.tile([P, CH], F32, tag="m")
                        for g in range(kg):
                            nc.tensor.matmul(
                                m_ps[:],
                                lhsT_t[:, g * NPAD + b * P:g * NPAD + (b + 1) * P],
                                wt[:, g * CH:(g + 1) * CH],
                                start=(g == 0), stop=(g == kg - 1))
                        m_bf = mp.tile([P, CH], BF, tag="mbf")
                        nc.vector.tensor_copy(m_bf[:], m_ps[:])
                        nc.sync.dma_start(m_dram[b * P:(b + 1) * P, :], m_bf[:])

                    if l == 0 and rep == 0:
                        # load late constants while layer-0 messages run
                        nc.sync.dma_start(wk_t[:], wk[:])
                        nc.sync.dma_start(vk_t[:], vk[:])
                        nc.sync.dma_start(wd_t[:], wd[:])
                        nc.sync.dma_start(bdr_t[:], bdr[:])

                    # --- partial aggregates for all 80 global src blocks
                    rs_in = dram.tile([NCORES, NBL, P, CH], BF, tag="rs_in")
                    # batched gathers: GBATCH chunks per indirect DMA
                    msg_tiles = {}
                    for c0 in range(0, neb, GBATCH):
                        cw = min(GBATCH, neb - c0)
                        mt = msgp.tile([P, GBATCH * CH], BF, tag="msg")
                        nc.gpsimd.indirect_dma_start(
                            out=mt[:, :cw * CH], out_offset=None,
                            in_=m_dram[:],
                            in_offset=bass.IndirectOffsetOnAxis(
                                ap=idx_t[:, c0:c0 + cw], axis=0))
                        msg_tiles[c0] = mt
                    col = 0
                    for oi, b in enumerate(GB_ORDER):
                        agg_ps = psagg.tile([P, CH], F32, tag="agg")
                        for k in range(kcb[oi]):
                            mt = msg_tiles[(col // GBATCH) * GBATCH]
                            j = col % GBATCH
                            nc.tensor.matmul(
                                agg_ps[:],
                                ctab_t[:, col * P:(col + 1) * P],
                                mt[:, j * CH:(j + 1) * CH],
                                start=(k == 0), stop=(k == kcb[oi] - 1))
                            col += 1
                        agg_bf = aggfp.tile([P, CH], BF, tag="aggbf")
                        nc.vector.tensor_copy(agg_bf[:], agg_ps[:])
                        nc.sync.dma_start(rs_in[b // NBL, b % NBL, :, :], agg_bf[:])

                    # --- ReduceScatter: sum partials, keep own rows
                    rs_out = dram.tile([NBL, P, CH], BF, tag=f"rs_out{l}")
                    nc.gpsimd.collective_compute(
                        "ReduceScatter", mybir.AluOpType.add,
                        replica_groups=[list(range(NCORES))],
                        ins=[rs_in[:].opt()], outs=[rs_out[:].opt()])

                    # --- h' = relu(agg + h@V + b); transpose for next layer
                    hT_next = htp.tile([P, KGC * NPAD], BF, tag="hT")
                    for b in range(NBL):
                        agg_sb = hp.tile([P, CH], BF, tag="aggsb")
                        nc.sync.dma_start(agg_sb[:], rs_out[b, :, :])
                        h_ps = psagg.tile([P, CH], F32, tag="agg")
                        nc.tensor.matmul(
                            h_ps[:], ident[:], agg_sb[:],
                            start=True, stop=False)
                        for g in range(kg):
                            nc.tensor.matmul(
                                h_ps[:],
                                lhsT_t[:, g * NPAD + b * P:g * NPAD + (b + 1) * P],
                                vt[:, g * CH:(g + 1) * CH],
                                start=False, stop=(g == kg - 1))
                        nc.vector.tensor_add(
                            h_ps[:], h_ps[:], ball_t[:, l * CH:(l + 1) * CH])
                        h_bf = hp.tile([P, CH], BF, tag="h")
                        nc.scalar.activation(
                            h_bf[:], h_ps[:], mybir.ActivationFunctionType.Relu)
                        for cg in range(KGC):
                            tr_ps = pstr.tile([P, P], BF, tag="tr")
                            nc.tensor.transpose(
                                tr_ps[:], h_bf[:, cg * P:(cg + 1) * P], ident[:])
                            nc.vector.tensor_copy(
                                hT_next[:, cg * NPAD + b * P:cg * NPAD + (b + 1) * P],
                                tr_ps[:])
                    hT_cur = hT_next

                # ---- final dense: logits = h7 @ Wd + bd
                for b in range(NBL):
                    for c in range(3):
                        fin_ps = psagg.tile([P, FIN_CHUNK], F32, tag="agg")
                        for g in range(KGC):
                            nc.tensor.matmul(
                                fin_ps[:],
                                hT_cur[:, g * NPAD + b * P:g * NPAD + (b + 1) * P],
                                wd_t[:, g * N_LABELS + c * FIN_CHUNK:
                                     g * N_LABELS + (c + 1) * FIN_CHUNK],
                                start=(g == 0), stop=(g == KGC - 1))
                        nc.vector.tensor_add(
                            fin_ps[:], fin_ps[:],
                            bdr_t[:, c * FIN_CHUNK:(c + 1) * FIN_CHUNK])
                        o_sb = op.tile([P, FIN_CHUNK], F32, tag="o")
                        nc.scalar.activation(
                            o_sb[:], fin_ps[:],
                            mybir.ActivationFunctionType.Copy)
                        if rep == repeat - 1:
                            nc.sync.dma_start(
                                out[b * P:(b + 1) * P,
                                    c * FIN_CHUNK:(c + 1) * FIN_CHUNK],
                                o_sb[:])

    _split_excess_waits(nc)
    return nc


# ------------------------------------------------------------- entry point
def kernel(x, src, dst, W1, V1, b1, Wk, Vk, bk, Wd, bd, _repeat=1, _nc_cache={}):
    x = np.asarray(x, np.float32)
    kcb, idx_tabs, c_tabs = _prep_edges(src, dst)

    key = (tuple(kcb), _repeat)
    if key not in _nc_cache:
        _nc_cache[key] = _build(kcb, repeat=_repeat)
    nc = _nc_cache[key]

    # weights (replicated, host-packed)
    w1p = _pack_rhs(np.asarray(W1, np.float32), KG1, CH).astype(BFNP)
    v1p = _pack_rhs(np.asarray(V1, np.float32), KG1, CH).astype(BFNP)
    wkp = np.concatenate(
        [_pack_rhs(np.asarray(Wk[i], np.float32), KGC, CH) for i in range(6)],
        axis=1).astype(BFNP)
    vkp = np.concatenate(
        [_pack_rhs(np.asarray(Vk[i], np.float32), KGC, CH) for i in range(6)],
        axis=1).astype(BFNP)
    wdp = _pack_rhs(np.asarray(Wd, np.float32), KGC, N_LABELS).astype(BFNP)
    ballv = np.concatenate(
        [np.asarray(b1, np.float32)] + [np.asarray(bk[i], np.float32)
                                        for i in range(6)])
    ballp = np.broadcast_to(ballv, (P, NLAYERS * CH)).copy()
    bdp = np.broadcast_to(np.asarray(bd, np.float32), (P, N_LABELS)).copy()

    in_maps = []
    for p in range(NCORES):
        xp = np.zeros((NPAD, IN_F), np.float32)
        xp[:NPC] = x[p * NPC:(p + 1) * NPC]
        xTp = _pack_lhsT(np.ascontiguousarray(xp.T), KG1).astype(BFNP)
        in_maps.append({
            "xT": xTp, "idx": idx_tabs[p], "ctab": c_tabs[p],
            "w1": w1p, "v1": v1p, "wk": wkp, "vk": vkp, "wd": wdp,
            "ball": ballp, "bdr": bdp,
        })

    res = run_bass_kernel_spmd(nc, in_maps, core_ids=list(range(NCORES)))
    outp = np.empty((N_NODES, N_LABELS), np.float32)
    for p in range(NCORES):
        outp[p * NPC:(p + 1) * NPC] = res.results[p]["out"][:NPC]
    return outp


# revision 12
# speedup vs baseline: 2.2012x; 1.9618x over previous
"""ARMA-style GNN message passing on 8 TRN2 NeuronCores.

Reference computation (per layer, 7 layers):
    m   = h @ W                                  [N, CH]
    agg = segment_sum(w[:,None] * m[dst], src)   [N, CH]
    h'  = relu(agg + h @ V + b)
then logits = h @ Wd + bd.

Strategy (graph/data parallel over nodes, ReduceScatter aggregation):
  - 8 cores own 1250 nodes each (padded to 1280 = 10 blocks of 128).
  - Edge (s, d) is processed by the core owning d (the message SOURCE row):
    core q computes m for its own nodes only, keeps it local, and produces
    PARTIAL aggregates for ALL 80 global src node-blocks from its own edges.
    A ReduceScatter(add) then sums the 8 partials and hands each core the
    aggregate rows for its own nodes.  Output of the collective is only
    [1280, 512] bf16 per core (vs a 10.5 MB AllGather), and all message
    gathers are core-local.
  - Per global src block b, the host dedups the edges' dst rows (unique
    rows only), builds gather index chunks of 128 rows and [128 x 128]
    bf16 "C" matrices with the summed degree weights; segment-sum becomes
    C.T @ gathered_rows on PE, accumulated in PSUM over the block's chunks.
  - Gathers are batched (8 chunks per indirect DMA) to amortize the SWDGE
    fixed overhead.
  - h@V accumulates into the same PSUM bank as the scattered aggregate
    (identity-matmul injects the ReduceScatter result); bias + relu +
    PE-transpose produce the next layer's stationary operand hT.
  - Final dense layer and output assembly per core; host concatenates.

All matmuls run in bf16 with fp32 PSUM accumulation.
"""
import numpy as np
import ml_dtypes

import concourse.bass as bass
import concourse.tile as tile
import concourse.mybir as mybir
import bass_rust as _bass_rust
from concourse.vector_clock import ScopedClock
from concourse.bass_utils import run_bass_kernel_spmd
from concourse.masks import make_identity
from concourse import library_config

# ---------------------------------------------------------------- constants
N_NODES = 10000
N_EDGES = 160000
IN_F = 256
CH = 512
N_LABELS = 1440
NCORES = 8
NPC = N_NODES // NCORES      # 1250 nodes per core
P = 128
NBL = 10                     # node blocks per core (10*128 = 1280)
NPAD = NBL * P               # padded nodes per core
GBL = NCORES * NBL           # 80 global src node blocks
NLAYERS = 7
KG1 = IN_F // P              # 2 contraction blocks in layer 1
KGC = CH // P                # 4 contraction blocks in layers 2..7
FIN_CHUNK = 480              # 1440 = 3 * 480, fits one PSUM bank in f32
GBATCH = 8                   # gather chunks batched per dma_gather call

BF = mybir.dt.bfloat16
F32 = mybir.dt.float32
BFNP = ml_dtypes.bfloat16

# processing order of global src blocks: j-major so that each core's
# low-j blocks complete first (enables split collectives)
GB_ORDER = [p * NBL + j for j in range(NBL) for p in range(NCORES)]


# ------------------------------------------------------- walrus workarounds
def _patched_drain_and_barrier(self, tick_clock, wait_clock):
    # This walrus build rejects >1-2 sync waits on one TPB_CTRL; put the
    # kernel-tail drain's waits on separate preceding SP nops instead.
    nc = self.nc
    probe = nc.sync.nop(nofuse=True, hint="drain_waits")
    wait_clock.add_sem_waits(probe.ins, ScopedClock({None: tick_clock.global_clock}))
    si = probe.ins.sync_info
    waits = list(si.on_wait) if si is not None else []
    if len(waits) > 1:
        si.on_wait = waits[:1]
        for i in range(1, len(waits)):
            n2 = nc.sync.nop(nofuse=True, hint=f"drain_waits_{i}")
            n2.ins.sync_info = mybir.SyncInfo(on_wait=[waits[i]], on_update=[])
    nc.sync.drain()
    nc.all_engine_barrier()
    assert self.sems is not None
    popped = nc._tile_sem_poison_stack.pop()
    assert popped is self._sem_poison
    nc.clear_and_free_semaphores(list(self.sems.allocated().values()))
    nc.all_engine_barrier()


tile.TileContext._drain_and_barrier = _patched_drain_and_barrier


def _split_excess_waits(nc, limit=1):
    # Same ISA restriction for ordinary instructions: hoist excess sync
    # waits onto injected same-engine nops placed just before.
    for func in nc.m.functions:
        for bb in func.blocks:
            out = []
            for ins in bb.instructions:
                si = ins.sync_info
                if si is not None and si.on_wait and len(si.on_wait) > limit:
                    waits = list(si.on_wait)
                    excess, keep = waits[:-limit], waits[-limit:]
                    for i in range(0, len(excess), limit):
                        out.append(mybir.InstNoOp(
                            name=f"{ins.name}_xw{i}",
                            engine=ins.engine,
                            ins=[], outs=[],
                            sync_info=mybir.SyncInfo(
                                on_wait=excess[i:i + limit], on_update=[]),
                        ))
                    si.on_wait = keep
                out.append(ins)
            bb.instructions[:] = out


# ------------------------------------------------------------- host prep
def _prep_edges(src, dst):
    """Partition edges by owning core of dst; per (core, global src block)
    dedup dst rows and build gather-index chunks + C matrices (summed
    degree weights).  Returns (kcb, idx_tabs, c_tabs): kcb[i] = chunk count
    of the i-th block in GB_ORDER (same on all cores)."""
    src = np.asarray(src).astype(np.int64)
    dst = np.asarray(dst).astype(np.int64)
    deg_out = np.maximum(np.bincount(src, minlength=N_NODES), 1.0).astype(np.float64)
    deg_in = np.maximum(np.bincount(dst, minlength=N_NODES), 1.0).astype(np.float64)
    w = (1.0 / np.sqrt(deg_out[src] * deg_in[dst])).astype(np.float32)

    core = dst // NPC                       # owner of the message source row
    ldst = dst - core * NPC                 # local gather row
    sloc = src % NPC
    gb = (src // NPC) * NBL + sloc // P     # global src block
    slot = sloc % P                         # column within the block

    # bucket edges by (core, gb)
    order = np.lexsort((gb, core))
    core_s, gb_s = core[order], gb[order]
    ldst_s, slot_s, w_s = ldst[order], slot[order], w[order]
    counts = np.zeros((NCORES, GBL), np.int64)
    np.add.at(counts, (core_s, gb_s), 1)
    starts = np.zeros((NCORES, GBL), np.int64)
    flat = counts.ravel().cumsum()
    starts.ravel()[1:] = flat[:-1]

    # unique-dst rows per (core, gb)
    uniq = [[None] * GBL for _ in range(NCORES)]
    nuniq = np.zeros((NCORES, GBL), np.int64)
    inv_all = [[None] * GBL for _ in range(NCORES)]
    for q in range(NCORES):
        for b in range(GBL):
            s0, cnt = starts[q, b], counts[q, b]
            u, inv = np.unique(ldst_s[s0:s0 + cnt], return_inverse=True)
            uniq[q][b] = u
            inv_all[q][b] = inv
            nuniq[q, b] = len(u)

    kcb = [max(1, int(-(-nuniq[:, b].max() // P))) for b in GB_ORDER]
    neb = sum(kcb)

    idx_tabs, c_tabs = [], []
    for q in range(NCORES):
        # chunk c's 128 gather rows live at int16 columns c*8..c*8+7:
        # within-call index i = (c-c0)*128 + p  ->  [i%16, i//16]
        # = [p%16, (c-c0)*8 + p//16]  (replicated across the 8 stripes)
        idx_t = np.zeros((P, neb * (P // 16)), np.int16)
        c_t = np.zeros((P, neb * P), np.float32)
        col = 0
        for oi, b in enumerate(GB_ORDER):
            s0, cnt = starts[q, b], counts[q, b]
            u = uniq[q][b]
            inv = inv_all[q][b]          # edge -> position in u
            sl = slot_s[s0:s0 + cnt]
            ww = w_s[s0:s0 + cnt]
            for k in range(kcb[oi]):
                lo, hi = k * P, min((k + 1) * P, len(u))
                if hi > lo:
                    rows = np.zeros(P, np.int16)
                    rows[:hi - lo] = u[lo:hi]
                    pp = np.arange(P)
                    for s in range(P // 16):
                        idx_t[s * 16 + pp % 16, col * (P // 16) + pp // 16] = rows
                    sel = (inv >= lo) & (inv < hi)
                    np.add.at(c_t, (inv[sel] - lo, col * P + sl[sel]), ww[sel])
                col += 1
        idx_tabs.append(idx_t)
        c_tabs.append(c_t.astype(BFNP))
    return kcb, idx_tabs, c_tabs


def _pack_lhsT(xT, kg):
    """[kg*128, NPAD] -> [128, kg*NPAD] (partition-major kg blocks)."""
    return np.ascontiguousarray(
        xT.reshape(kg, P, NPAD).transpose(1, 0, 2).reshape(P, kg * NPAD))


def _pack_rhs(Wm, kg, n):
    """[kg*128, n] -> [128, kg*n]."""
    return np.ascontiguousarray(
        Wm.reshape(kg, P, n).transpose(1, 0, 2).reshape(P, kg * n))


# ------------------------------------------------------------- device build
def _build(kcb, repeat=1):
    neb = sum(kcb)
    nc = bass.Bass("TRN2", target_bir_lowering=False, debug=False,
                   num_devices=NCORES)

    def din(name, shape, dt):
        return nc.dram_tensor(name, shape, dt, kind="ExternalInput").ap()

    xT = din("xT", [P, KG1 * NPAD], BF)
    idx = din("idx", [P, neb * (P // 16)], mybir.dt.int16)
    ctab = din("ctab", [P, neb * P], BF)
    w1 = din("w1", [P, KG1 * CH], BF)
    v1 = din("v1", [P, KG1 * CH], BF)
    wk = din("wk", [P, 6 * KGC * CH], BF)
    vk = din("vk", [P, 6 * KGC * CH], BF)
    wd = din("wd", [P, KGC * N_LABELS], BF)
    ball = din("ball", [P, NLAYERS * CH], F32)
    bdr = din("bdr", [P, N_LABELS], F32)
    out = nc.dram_tensor("out", [NPAD, N_LABELS], F32, kind="ExternalOutput").ap()

    # chunk -> ordered-block mapping
    chunk_block = []          # index into GB_ORDER position for each chunk col
    for oi in range(GBL):
        chunk_block.extend([oi] * kcb[oi])

    with tile.TileContext(nc) as tc:
        with (
            tc.tile_pool(name="const", bufs=1) as cp,
            tc.tile_pool(name="ht", bufs=2) as htp,
            tc.tile_pool(name="mout", bufs=3) as mp,
            tc.tile_pool(name="msg", bufs=2) as msgp,
            tc.tile_pool(name="aggf", bufs=4) as aggfp,
            tc.tile_pool(name="hact", bufs=2) as hp,
            tc.tile_pool(name="outs", bufs=3) as op,
            tc.tile_pool(name="psm", bufs=2, space="PSUM") as psm,
            tc.tile_pool(name="psagg", bufs=4, space="PSUM") as psagg,
            tc.tile_pool(name="pstr", bufs=2, space="PSUM") as pstr,
            tc.tile_pool(name="dram", bufs=1, space="DRAM") as dram,
        ):
            # ---- constants needed for layer 0 first
            nc.gpsimd.load_library(library_config.mlp)
            # shared num_idxs registers (one per distinct batch width)
            nidx_regs = {}
            for cw in {GBATCH, neb % GBATCH or GBATCH}:
                nidx_regs[cw] = nc.gpsimd.to_reg(cw * P)
            xT_t = cp.tile([P, KG1 * NPAD], BF)
            nc.sync.dma_start(xT_t[:], xT[:])
            w1_t = cp.tile([P, KG1 * CH], BF)
            nc.sync.dma_start(w1_t[:], w1[:])
            v1_t = cp.tile([P, KG1 * CH], BF)
            nc.sync.dma_start(v1_t[:], v1[:])
            idx_t = cp.tile([P, neb * (P // 16)], mybir.dt.int16)
            nc.sync.dma_start(idx_t[:], idx[:])
            ctab_t = cp.tile([P, neb * P], BF)
            nc.sync.dma_start(ctab_t[:], ctab[:])
            ball_t = cp.tile([P, NLAYERS * CH], F32)
            nc.sync.dma_start(ball_t[:], ball[:])
            ident = cp.tile([P, P], BF)
            make_identity(nc, ident[:])
            # late constants (first needed at layer 1 / final layer)
            wk_t = cp.tile([P, 6 * KGC * CH], BF)
            vk_t = cp.tile([P, 6 * KGC * CH], BF)
            wd_t = cp.tile([P, KGC * N_LABELS], BF)
            bdr_t = cp.tile([P, N_LABELS], F32)

            for rep in range(repeat):
                hT_cur = None
                for l in range(NLAYERS):
                    kg = KG1 if l == 0 else KGC
                    if l == 0:
                        lhsT_t = xT_t
                        wt = w1_t[:, :]
                        vt = v1_t[:, :]
                    else:
                        lhsT_t = hT_cur
                        wt = wk_t[:, (l - 1) * KGC * CH:l * KGC * CH]
                        vt = vk_t[:, (l - 1) * KGC * CH:l * KGC * CH]

                    # --- m = h @ W for own nodes; stage to local DRAM
                    m_dram = dram.tile([NPAD, CH], BF, tag="m_dram")
                    for b in range(NBL):
                        m_ps = psm.tile([P, CH], F32, tag="m")
                        for g in range(kg):
                            nc.tensor.matmul(
                                m_ps[:],
                                lhsT_t[:, g * NPAD + b * P:g * NPAD + (b + 1) * P],
                                wt[:, g * CH:(g + 1) * CH],
                                start=(g == 0), stop=(g == kg - 1))
                        m_bf = mp.tile([P, CH], BF, tag="mbf")
                        nc.vector.tensor_copy(m_bf[:], m_ps[:])
                        nc.sync.dma_start(m_dram[b * P:(b + 1) * P, :], m_bf[:])

                    if l == 0 and rep == 0:
                        # load late constants while layer-0 messages run
                        nc.sync.dma_start(wk_t[:], wk[:])
                        nc.sync.dma_start(vk_t[:], vk[:])
                        nc.sync.dma_start(wd_t[:], wd[:])
                        nc.sync.dma_start(bdr_t[:], bdr[:])

                    # --- partial aggregates for all 80 global src blocks
                    rs_in = dram.tile([NCORES, NBL, P, CH], BF, tag="rs_in")
                    # batched gathers: GBATCH chunks per dma_gather call
                    msg_tiles = {}
                    for c0 in range(0, neb, GBATCH):
                        cw = min(GBATCH, neb - c0)
                        mt = msgp.tile([P, GBATCH * CH], BF, tag="msg")
                        nreg = nidx_regs[cw]
                        nc.gpsimd.dma_gather(
                            mt[:].rearrange("p (k e) -> p k e", e=CH)[:, :cw, :],
                            m_dram[:],
                            idx_t[:, c0 * (P // 16):(c0 + cw) * (P // 16)],
                            cw * P, nreg, CH)
                        msg_tiles[c0] = mt
                    col = 0
                    for oi, b in enumerate(GB_ORDER):
                        agg_ps = psagg.tile([P, CH], F32, tag="agg")
                        for k in range(kcb[oi]):
                            mt = msg_tiles[(col // GBATCH) * GBATCH]
                            j = col % GBATCH
                            nc.tensor.matmul(
                                agg_ps[:],
                                ctab_t[:, col * P:(col + 1) * P],
                                mt[:, j * CH:(j + 1) * CH],
                                start=(k == 0), stop=(k == kcb[oi] - 1))
                            col += 1
                        agg_bf = aggfp.tile([P, CH], BF, tag="aggbf")
                        nc.vector.tensor_copy(agg_bf[:], agg_ps[:])
                        nc.sync.dma_start(rs_in[b // NBL, b % NBL, :, :], agg_bf[:])

                    # --- ReduceScatter: sum partials, keep own rows
                    rs_out = dram.tile([NBL, P, CH], BF, tag=f"rs_out{l}")
                    nc.gpsimd.collective_compute(
                        "ReduceScatter", mybir.AluOpType.add,
                        replica_groups=[list(range(NCORES))],
                        ins=[rs_in[:].opt()], outs=[rs_out[:].opt()])

                    # --- h' = relu(agg + h@V + b); transpose for next layer
                    hT_next = htp.tile([P, KGC * NPAD], BF, tag="hT")
                    for b in range(NBL):
                        agg_sb = hp.tile([P, CH], BF, tag="aggsb")
                        nc.sync.dma_start(agg_sb[:], rs_out[b, :, :])
                        h_ps = psagg.tile([P, CH], F32, tag="agg")
                        nc.tensor.matmul(
                            h_ps[:], ident[:], agg_sb[:],
                            start=True, stop=False)
                        for g in range(kg):
                            nc.tensor.matmul(
                                h_ps[:],
                                lhsT_t[:, g * NPAD + b * P:g * NPAD + (b + 1) * P],
                                vt[:, g * CH:(g + 1) * CH],
                                start=False, stop=(g == kg - 1))
                        nc.vector.tensor_add(
                            h_ps[:], h_ps[:], ball_t[:, l * CH:(l + 1) * CH])
                        h_bf = hp.tile([P, CH], BF, tag="h")
                        nc.scalar.activation(
                            h_bf[:], h_ps[:], mybir.ActivationFunctionType.Relu)
                        for cg in range(KGC):
                            tr_ps = pstr.tile([P, P], BF, tag="tr")
                            nc.tensor.transpose(
                                tr_ps[:], h_bf[:, cg * P:(cg + 1) * P], ident[:])
                            nc.vector.tensor_copy(
                                hT_next[:, cg * NPAD + b * P:cg * NPAD + (b + 1) * P],
                                tr_ps[:])
                    hT_cur = hT_next

                # ---- final dense: logits = h7 @ Wd + bd
                for b in range(NBL):
                    for c in range(3):
                        fin_ps = psagg.tile([P, FIN_CHUNK], F32, tag="agg")
                        for g in range(KGC):
                            nc.tensor.matmul(
                                fin_ps[:],
                                hT_cur[:, g * NPAD + b * P:g * NPAD + (b + 1) * P],
                                wd_t[:, g * N_LABELS + c * FIN_CHUNK:
                                     g * N_LABELS + (c + 1) * FIN_CHUNK],
                                start=(g == 0), stop=(g == KGC - 1))
                        nc.vector.tensor_add(
                            fin_ps[:], fin_ps[:],
                            bdr_t[:, c * FIN_CHUNK:(c + 1) * FIN_CHUNK])
                        o_sb = op.tile([P, FIN_CHUNK], F32, tag="o")
                        nc.scalar.activation(
                            o_sb[:], fin_ps[:],
                            mybir.ActivationFunctionType.Copy)
                        if rep == repeat - 1:
                            nc.sync.dma_start(
                                out[b * P:(b + 1) * P,
                                    c * FIN_CHUNK:(c + 1) * FIN_CHUNK],
                                o_sb[:])

    _split_excess_waits(nc)
    # lower extension instructions (dma_gather): insert GPSIMD library
    # loads and populate .instr bytes (else walrus fails "ISA wrong length")
    mask = {}
    for lib in library_config.all_libraries:
        for it in lib.instructions:
            mask[it] = mask.get(it, 0) | (1 << lib.index)
    _bass_rust.insert_library_loads(
        nc, mask, len(library_config.all_libraries), library_config.standard.index)
    mybir.codegen_inst_isa_subclasses(nc)
    return nc


# ------------------------------------------------------------- entry point
def kernel(x, src, dst, W1, V1, b1, Wk, Vk, bk, Wd, bd, _repeat=1, _nc_cache={}):
    x = np.asarray(x, np.float32)
    kcb, idx_tabs, c_tabs = _prep_edges(src, dst)

    key = (tuple(kcb), _repeat)
    if key not in _nc_cache:
        _nc_cache[key] = _build(kcb, repeat=_repeat)
    nc = _nc_cache[key]

    # weights (replicated, host-packed)
    w1p = _pack_rhs(np.asarray(W1, np.float32), KG1, CH).astype(BFNP)
    v1p = _pack_rhs(np.asarray(V1, np.float32), KG1, CH).astype(BFNP)
    wkp = np.concatenate(
        [_pack_rhs(np.asarray(Wk[i], np.float32), KGC, CH) for i in range(6)],
        axis=1).astype(BFNP)
    vkp = np.concatenate(
        [_pack_rhs(np.asarray(Vk[i], np.float32), KGC, CH) for i in range(6)],
        axis=1).astype(BFNP)
    wdp = _pack_rhs(np.asarray(Wd, np.float32), KGC, N_LABELS).astype(BFNP)
    ballv = np.concatenate(
        [np.asarray(b1, np.float32)] + [np.asarray(bk[i], np.float32)
                                        for i in range(6)])
    ballp = np.broadcast_to(ballv, (P, NLAYERS * CH)).copy()
    bdp = np.broadcast_to(np.asarray(bd, np.float32), (P, N_LABELS)).copy()

    in_maps = []
    for p in range(NCORES):
        xp = np.zeros((NPAD, IN_F), np.float32)
        xp[:NPC] = x[p * NPC:(p + 1) * NPC]
        xTp = _pack_lhsT(np.ascontiguousarray(xp.T), KG1).astype(BFNP)
        in_maps.append({
            "xT": xTp, "idx": idx_tabs[p], "ctab": c_tabs[p],
            "w1": w1p, "v1": v1p, "wk": wkp, "vk": vkp, "wd": wdp,
            "ball": ballp, "bdr": bdp,
        })

    res = run_bass_kernel_spmd(nc, in_maps, core_ids=list(range(NCORES)))
    outp = np.empty((N_NODES, N_LABELS), np.float32)
    for p in range(NCORES):
        outp[p * NPC:(p + 1) * NPC] = res.results[p]["out"][:NPC]
    return outp
